# revision 8
# baseline (speedup 1.0000x reference)
"""Trainium2 Bass kernel for nn_DGraFormer_framework (gnn_message_passing).

Reference computation (B=32, N=64, S=336, D=32, K=3 layers, beta=0.05):
    per (b, s):  A = adj[b,s]  (row-normalized [N,N])
    H0 = x w_start + b_start          [N, D]
    H_{k+1} = beta*x + (1-beta) A^T H_k
    out = concat(H_0..H_3) @ w_mlp + b_mlp   -> [b, n, s]

Everything is linear in the feature dim, so D collapses:
    out[b,:,s] = pre0 + A'(pre1 + A'(pre2 + A' pre3))      (Horner)
where A' = A^T and pre_j[b,n,s] = c_j * x[b,n,s] + d_j (scalars c_j, d_j, e
derived from w_start/b_start/w_mlp/b_mlp on the host; e folded into pre0).

Quantization: adj is stored fp8 (e3m4) scaled by 16. pre_j planes are
pre-scaled by 16^(3-j) on the host and the final output is divided by 16^3
after the gather.

Device kernel (per core; data-parallel over batch, 4 b per core):
  - adj[b] packed as 84 "quads": 4 A-matrices per 128x128 stationary tile
    (2x2 blocks of 64x64), fp8 e3m4.  Quad block (pb,cb) holds
    A_{s=4q+sigma(pb,cb)}, sigma = [[1,0],[2,3]][pb][cb]; chain s=4q+j has
    its vector in input half (j0,j1 top; j2,j3 bottom) and its result at
    the class output half (j1 top/top, j3 bot/bot direct; j0 top->bot,
    j2 bot->top crossed).
  - All chain-vector/psum/pre tensors use a CLASS-PLANE column layout:
    col = j*84 + q (plane-major), so per-class data is contiguous.  The
    matmul moving/output APs read/write the 4 planes at stride 84.
  - 3 passes of one matmul per quad.  Pass transitions:
      * direct classes (j1, j3): one DVE add psum+pre -> V  each.
      * crossed classes (j0, j2): one DVE add stages psum+pre into TA,
        then two contiguous [64, 84] SBUF->SBUF DMAs on the Activation
        hardware-DGE queue move the halves into V.  No PE, no Scalar.
  - All tiles are per-batch: adj, host-shipped pre planes (pre0..pre2
    mirrored + v3 start vectors, [128, 4S] fp16 per batch), V1/V2 chain
    tiles, TA staging, TF output staging.  PSUM: 8 rotating pass slots.
  - DMAs ride the two hardware DGE queues (SP: adj + outputs; Activation:
    pre planes + transition swaps).  gpsimd only does V-tile memsets.
  - Final pass lands psum+pre0 in TF[b] (class planes at output halves);
    host reassembles (free) after the gather.
"""

import sys

sys.path.insert(0, "/opt/trn_rl_repo")

import ml_dtypes
import numpy as np

import concourse.bass as bass
import concourse.mybir as mybir
import concourse.tile as tile
from concourse import bacc
from concourse.bass_utils import run_bass_kernel_spmd

B, N, S, D = 32, 64, 336, 32
MP_LAYERS = 3
PROPBETA = 0.05
NCORES = 8
BL = B // NCORES          # batches per core
Q = S // 4                # quads per batch (84)
H = Q // 2                # half split (42 quads)

ADJ_DT = mybir.dt.float8e3    # e3m4
ADJ_NP = ml_dtypes.float8_e3m4
ADJ_SCALE = 16.0
OUT_DESCALE = float(ADJ_SCALE ** MP_LAYERS)
V_DT = mybir.dt.float16       # chain-vector / pre / out dtype
V_NP = np.float16

f32 = mybir.dt.float32


def _coefficients(w_start, b_start, w_mlp, b_mlp):
    """Collapse the feature dim: out = sum_j A'^j (c_j x + d_j 1) + e (j=0..K)."""
    K = MP_LAYERS
    beta, sb = PROPBETA, 1.0 - PROPBETA
    ws = w_start[0].astype(np.float64)
    bs = b_start.astype(np.float64)
    w = [w_mlp[k * D:(k + 1) * D, 0].astype(np.float64) for k in range(K + 1)]

    u = {(0, 0): ws}
    v = {(0, 0): bs}
    for k in range(K):
        nu = {(k + 1, 0): beta * np.ones(D)}
        nv = {(k + 1, 0): np.zeros(D)}
        for j in range(k + 1):
            nu[(k + 1, j + 1)] = sb * u[(k, j)]
            nv[(k + 1, j + 1)] = sb * v[(k, j)]
        u.update(nu)
        v.update(nv)

    c = np.zeros(K + 1)
    d = np.zeros(K + 1)
    for k in range(K + 1):
        for j in range(k + 1):
            c[j] += float(u[(k, j)] @ w[k])
            d[j] += float(v[(k, j)] @ w[k])
    e = d[0] + float(b_mlp[0])
    return c, d, e


def build_nc():
    nc = bacc.Bacc("TRN2", target_bir_lowering=False, debug=False)

    adj_l = nc.dram_tensor("adj", [BL, 128, Q * 128], ADJ_DT,
                           kind="ExternalInput")
    # host-computed pre planes j=0,1,2 (mirrored over partition halves,
    # scaled by 16^(3-j), e folded into j=0) + the v3 start vectors, all
    # in class-plane column layout (col = f*84 + q)
    pre_l = nc.dram_tensor("prev3", [BL, 128, 4 * S], V_DT,
                           kind="ExternalInput")
    out_l = nc.dram_tensor("out", [BL, 128, S], V_DT, kind="ExternalOutput")

    with tile.TileContext(nc) as tc:
        with (
            tc.tile_pool(name="singles", bufs=1) as singles,
            tc.tile_pool(name="psb_pool", bufs=1, space=bass.MemorySpace.PSUM)
            as psb_pool,
        ):
            pre_t = [singles.tile([128, 4 * S], V_DT, tag=f"pre{b}",
                                  name=f"pre{b}") for b in range(BL)]
            V = {}
            for b in range(BL):
                for k in (1, 2):
                    V[b, k] = singles.tile([128, S], V_DT,
                                           tag=f"v{b}{k}", name=f"v{b}{k}")
            TA = [singles.tile([128, 2 * Q], V_DT, tag=f"ta{b}", name=f"ta{b}")
                  for b in range(BL)]
            TF = [singles.tile([128, S], V_DT, tag=f"tf{b}", name=f"tf{b}")
                  for b in range(BL)]
            adj_t = [singles.tile([128, Q * 128], ADJ_DT,
                                  tag=f"adj{b}", name=f"adj{b}")
                     for b in range(BL)]

            # ---- all input DMAs up front, hardware DGE queues only ----
            # SP queue: adj batch 0 in 3 growing chunks (earliest compute
            # start), then batches 1-3 in halves (the tile tracker gates
            # each pass-half on the whole DMA that wrote it).
            for q0, q1 in ((0, 12), (12, 36), (36, 84)):
                nc.sync.dma_start(out=adj_t[0][:, q0 * 128:q1 * 128],
                                  in_=adj_l[0][:, q0 * 128:q1 * 128])
            for b in (1, 2, 3):
                nc.sync.dma_start(out=adj_t[b][:, :H * 128],
                                  in_=adj_l[b][:, :H * 128])
                nc.sync.dma_start(out=adj_t[b][:, H * 128:],
                                  in_=adj_l[b][:, H * 128:])

            # Activation queue: batch 0's v3 slice alone first (the very
            # first matmul needs only it + adj chunk 0), then the planes.
            nc.scalar.dma_start(out=pre_t[0][:, 3 * S:],
                                in_=pre_l[0][:, 3 * S:])
            nc.scalar.dma_start(out=pre_t[0][:, :3 * S],
                                in_=pre_l[0][:, :3 * S])
            for b in (1, 2, 3):
                nc.scalar.dma_start(out=pre_t[b][:], in_=pre_l[b][:])

            # V chain tiles: complementary halves must be zero; memset the
            # whole tile once (gpsimd is otherwise idle, no dependencies)
            for b in range(BL):
                for k in (1, 2):
                    nc.gpsimd.memset(V[b, k][:], 0.0)

            def vsrc(b, k):
                if k == 3:
                    return pre_t[b][:, 3 * S:]
                return V[b, k][:]

            def pre_plane(b, j):
                return pre_t[b][:, j * S:(j + 1) * S]

            # PSUM: 8 rotating pass-psum slots [128, 336] (reuse distance
            # over 2 steps keeps psum WAR off the critical path)
            psb = [psb_pool.tile([128, 512], f32, tag=f"psb{i}",
                                 name=f"psb{i}") for i in range(8)]
            psums = {}
            _ctr = {"ps": 0}

            def ps_of(b, k):
                key = (b, k)
                if key not in psums:
                    s = _ctr["ps"] % 8
                    _ctr["ps"] += 1
                    psums[key] = psb[s][:, 0:336]
                return psums[key]

            def p_slice(b, k, q0, q1):
                # one matmul per quad; moving operand and psum output read/
                # write the 4 class planes at stride Q (col = j*Q + q)
                ps4 = ps_of(b, k).rearrange("p (j q) -> p j q", j=4)
                src4 = vsrc(b, k).rearrange("p (j q) -> p j q", j=4)
                for q in range(q0, q1):
                    nc.tensor.matmul(
                        ps4[:, :, q],
                        adj_t[b][:, 128 * q:128 * (q + 1)],
                        src4[:, :, q],
                        start=True, stop=True,
                    )

            def transition(b, k):
                # pass k psum -> V_{k-1} (class planes, full Q):
                #  direct classes j1 (top/top), j3 (bot/bot): DVE add
                #  crossed j0, j2: DVE stages psum+pre both halves into TA,
                #  then two contiguous SBUF->SBUF DMAs (Act hw queue) move
                #  the output halves into V's input halves
                ps = ps_of(b, k)
                pr = pre_plane(b, k - 1)
                vn = V[b, k - 1][:]
                nc.vector.tensor_add(vn[0:64, Q:2 * Q], ps[0:64, Q:2 * Q],
                                     pr[0:64, Q:2 * Q])
                nc.vector.tensor_add(vn[64:128, 3 * Q:], ps[64:128, 3 * Q:],
                                     pr[64:128, 3 * Q:])
                ps3 = ps.rearrange("p (j q) -> p j q", j=4)
                pr3 = pr.rearrange("p (j q) -> p j q", j=4)
                ta = TA[b][:, :].rearrange("p (t q) -> p t q", t=2)
                nc.vector.tensor_add(ta[:, :, :], ps3[:, 0:3:2, :],
                                     pr3[:, 0:3:2, :])
                # j0: output bottom -> input top; j2: output top -> bottom
                nc.scalar.dma_start(out=vn[0:64, 0:Q],
                                    in_=TA[b][64:128, 0:Q])
                nc.scalar.dma_start(out=vn[64:128, 2 * Q:3 * Q],
                                    in_=TA[b][0:64, Q:2 * Q])

            def f_stage(b):
                # final staging into TF class planes at the OUTPUT halves
                # (host picks the right half per class): planes (0,3) and
                # (1,2) in two DVE adds
                ps3 = ps_of(b, 1).rearrange("p (j q) -> p j q", j=4)
                pr3 = pre_plane(b, 0).rearrange("p (j q) -> p j q", j=4)
                tf3 = TF[b][:, :].rearrange("p (j q) -> p j q", j=4)
                nc.vector.tensor_add(tf3[:, 0:4:3, :], ps3[:, 0:4:3, :],
                                     pr3[:, 0:4:3, :])
                nc.vector.tensor_add(tf3[:, 1:3, :], ps3[:, 1:3, :],
                                     pr3[:, 1:3, :])

            # ---- 3-deep skewed software pipeline ---------------------------
            # step s: pass1(s-2) + out, pass2(s-1), pass3(s); transitions at
            # the end of the producing step (DVE + swap DMAs complete early
            # in the next step, before their consumers run mid-step)
            for s in range(BL + 2):
                c = s if s < BL else None
                b = s - 1 if 0 <= s - 1 < BL else None
                a = s - 2 if 0 <= s - 2 < BL else None

                if a is not None:
                    p_slice(a, 1, 0, H)
                    p_slice(a, 1, H, Q)
                    f_stage(a)
                    nc.sync.dma_start(out=out_l[a], in_=TF[a][:])
                if b is not None:
                    p_slice(b, 2, 0, H)
                    p_slice(b, 2, H, Q)
                    transition(b, 2)
                if c is not None:
                    if c == 0:
                        # paced to the adj chunk arrivals (12, 24, 48)
                        p_slice(0, 3, 0, 12)
                        p_slice(0, 3, 12, 36)
                        p_slice(0, 3, 36, Q)
                    else:
                        p_slice(c, 3, 0, H)
                        p_slice(c, 3, H, Q)
                    transition(c, 3)

    nc.finalize()
    return nc


_NC_CACHE = None


def _get_nc():
    global _NC_CACHE
    if _NC_CACHE is None:
        _NC_CACHE = build_nc()
    return _NC_CACHE


def _pack_adj(adj):
    """[B, S, N, N] f32 -> [B, 128, Q*128] fp8 (x16) quad layout."""
    sigma = np.array([[1, 0], [2, 3]])  # [pb][cb]
    s_idx = 4 * np.arange(Q)[:, None, None] + sigma[None, :, :]
    a = adj[:, s_idx]                      # [B, Q, 2pb, 2cb, n, m]
    a = a.transpose(0, 2, 4, 1, 3, 5)      # [B, pb, n, Q, cb, m]
    return np.ascontiguousarray(
        (a.reshape(B, 128, Q * 128) * ADJ_SCALE).astype(ADJ_NP))


def _to_planes(t):
    """[..., S] with s=4q+f -> [..., 4*Q] with col f*Q+q (class planes)."""
    sh = t.shape[:-1]
    return np.ascontiguousarray(
        t.reshape(*sh, Q, 4).swapaxes(-1, -2).reshape(*sh, 4 * Q))


def _prepare_in_maps(x, adj, w_start, b_start, w_mlp, b_mlp):
    c, d, e = _coefficients(np.asarray(w_start), np.asarray(b_start),
                            np.asarray(w_mlp), np.asarray(b_mlp))
    x = np.asarray(x, dtype=np.float32)
    adj = _pack_adj(np.asarray(adj, dtype=np.float32))
    # prev3[b]: planes j=0,1,2 (mirrored over partition halves, scaled by
    # 16^(3-j), e folded into j=0) + v3 start vectors at the input halves,
    # all in class-plane layout
    prev3 = np.zeros((B, 128, 4 * S), dtype=V_NP)
    for j in range(MP_LAYERS):
        sc = ADJ_SCALE ** (MP_LAYERS - j)
        plane = _to_planes((c[j] * sc * x + (e if j == 0 else d[j]) * sc)
                           .astype(V_NP))          # [B, N, 4Q]
        prev3[:, 0:64, j * S:(j + 1) * S] = plane
        prev3[:, 64:128, j * S:(j + 1) * S] = plane
    pre3 = _to_planes((c[MP_LAYERS] * x + d[MP_LAYERS]).astype(V_NP))
    v3 = np.zeros((B, 128, 4 * Q), dtype=V_NP)
    v3[:, 0:64, 0:2 * Q] = pre3[:, :, 0:2 * Q]          # j0, j1 top
    v3[:, 64:128, 2 * Q:] = pre3[:, :, 2 * Q:]          # j2, j3 bottom
    prev3[:, :, 3 * S:] = v3
    in_maps = []
    for i in range(NCORES):
        sl = slice(i * BL, (i + 1) * BL)
        in_maps.append({
            "adj": np.ascontiguousarray(adj[sl]),
            "prev3": np.ascontiguousarray(prev3[sl]),
        })
    return in_maps


def run_spmd(inputs, trace=False, **kw):
    in_maps = _prepare_in_maps(**inputs)
    res = run_bass_kernel_spmd(_get_nc(), in_maps,
                               core_ids=list(range(NCORES)), trace=trace, **kw)
    tfd = np.concatenate([r["out"] for r in res.results],
                         axis=0).astype(np.float32)
    # TF class planes, each valid at its OUTPUT half:
    # j0 -> bottom, j1 -> top, j2 -> top, j3 -> bottom
    out = np.empty((B, N, Q, 4), dtype=np.float32)
    out[..., 0] = tfd[:, 64:128, 0 * Q:1 * Q]
    out[..., 1] = tfd[:, 0:64, 1 * Q:2 * Q]
    out[..., 2] = tfd[:, 0:64, 2 * Q:3 * Q]
    out[..., 3] = tfd[:, 64:128, 3 * Q:4 * Q]
    return out.reshape(B, N, S) / OUT_DESCALE, res


def kernel(**inputs):
    out, _ = run_spmd(inputs)
    return out.astype(np.float32)


if __name__ == "__main__":
    # quick smoke test against a numpy oracle
    rng = np.random.default_rng(0)
    x = rng.standard_normal((B, N, S), dtype=np.float32)
    adj = rng.random((B, S, N, N), dtype=np.float32)
    adj /= adj.sum(-1, keepdims=True)
    w_start = rng.standard_normal((1, D)).astype(np.float32)
    b_start = (rng.standard_normal(D) * 0.01).astype(np.float32)
    w_mlp = (rng.standard_normal(((MP_LAYERS + 1) * D, 1)) /
             np.sqrt((MP_LAYERS + 1) * D)).astype(np.float32)
    b_mlp = (rng.standard_normal(1) * 0.01).astype(np.float32)

    got = kernel(x=x, adj=adj, w_start=w_start, b_start=b_start,
                 w_mlp=w_mlp, b_mlp=b_mlp)

    h = x[..., None] * w_start[0] + b_start
    outs = [h]
    a = np.transpose(adj, (0, 2, 3, 1))
    for _ in range(MP_LAYERS):
        conv = np.einsum('bnsc,bnms->bmsc', h, a, optimize=True)
        h = PROPBETA * x[..., None] + (1 - PROPBETA) * conv
        outs.append(h)
    hc = np.concatenate(outs, axis=-1)
    want = (hc @ w_mlp)[..., 0] + b_mlp[0]

    aerr = np.abs(got - want)
    print("max abs err:", aerr.max(),
          "normalized:", aerr.max() / np.abs(want).max())
